# revision 9
# baseline (speedup 1.0000x reference)
"""Trainium2 Bass kernel for the Dedicom decoder problem.

Math: with U = z * d (row-wise scale by the selected local_diag row),
    score_b = ((z[e0]*d) @ W) * d . z[e1] = U[e0] @ W @ U[e1]^T
so all-pairs scores S = (U @ W) @ U^T  ([N_DRUGS, N_DRUGS]) contain every
edge score.  We shard S by e0-block across the 8 cores: core c computes
S rows [512c, 512c+512) (~2.1 GF in bf16), streams them to DRAM, then a
256B-granular dma_gather pulls each edge's 128-wide candidate block and a
host-built one-hot mask + segmented reduce extracts the scalar, followed
by an on-chip sigmoid.  Edges are bucketed to cores by e0>>9 on the host;
results are scattered back to their original positions on the host.
"""

import numpy as np
import ml_dtypes

BF = ml_dtypes.bfloat16

N_DRUGS = 4096
D = 512
N_CORES = 8
BLK = N_DRUGS // N_CORES  # 512 rows of S per core
KC = D // 128             # 4 contraction chunks
MT = BLK // 128           # 4 row tiles of the core's S block
NCH = N_DRUGS // 512      # 8 column chunks of S
TPB = N_DRUGS // 128      # 32 tokens (128-wide blocks) per S row

_cache = {}


def _build(cap, dep_mode="helper", inplace=True, tail=True, gather_mode="real",
           ms_load=True):
    """Build + compile the SPMD program for a per-core edge capacity `cap`."""
    import concourse.bass as bass  # noqa: F401
    import concourse.bacc as bacc
    import concourse.mybir as mybir
    import concourse.tile as tile
    from concourse.tile import add_dep_helper

    f32 = mybir.dt.float32
    bf16 = mybir.dt.bfloat16
    i16 = mybir.dt.int16
    nblk = cap // 128

    nc = bacc.Bacc("TRN2", target_bir_lowering=False, debug=False,
                   num_devices=N_CORES, dynamic_dma_scratch_size=65536)

    ZT = nc.dram_tensor("zt", [D, N_DRUGS], bf16, kind="ExternalInput")
    ZB = nc.dram_tensor("zb", [D, BLK], bf16, kind="ExternalInput")
    WT = nc.dram_tensor("w", [D, D], bf16, kind="ExternalInput")
    DT = nc.dram_tensor("dvec", [128, KC], f32, kind="ExternalInput")
    MS = nc.dram_tensor("mask", [128, nblk, 128], bf16, kind="ExternalInput")
    IX = nc.dram_tensor("idx", [128, cap // 16], i16, kind="ExternalInput")
    OUT = nc.dram_tensor("out", [128, nblk], f32, kind="ExternalOutput")
    SD = nc.dram_tensor("s_scratch", [BLK, N_DRUGS], bf16)

    with tile.TileContext(nc) as tc:
        with (
            tc.tile_pool(name="big", bufs=1) as big,
            tc.tile_pool(name="sml", bufs=1) as sml,
            tc.tile_pool(name="stage", bufs=4) as stage,
            tc.tile_pool(name="psum", bufs=8, space="PSUM") as psum,
        ):
            d_sb = sml.tile([128, KC], f32)
            nc.sync.dma_start(d_sb[:], DT.ap())
            w_sb = sml.tile([128, KC, D], bf16)
            nc.sync.dma_start(w_sb[:], WT.ap().rearrange("(jc p) k -> p jc k", p=128))
            zb_sb = sml.tile([128, KC, BLK], bf16)
            nc.sync.dma_start(zb_sb[:], ZB.ap().rearrange("(kc p) m -> p kc m", p=128))
            zt_sb = big.tile([128, KC, N_DRUGS], bf16)
            nc.sync.dma_start(zt_sb[:], ZT.ap().rearrange("(kc p) n -> p kc n", p=128))

            # U^T = z^T * d  (d is a per-partition scalar in each K chunk)
            for kc in range(KC):
                nc.vector.tensor_scalar_mul(zb_sb[:, kc, :], zb_sb[:, kc, :],
                                            d_sb[:, kc:kc + 1])
                nc.vector.tensor_scalar_mul(zt_sb[:, kc, :], zt_sb[:, kc, :],
                                            d_sb[:, kc:kc + 1])

            # A^T chunks for this core's block: a_sb[p, kc, m] = (U@W)[m, kc*128+p]
            a_sb = sml.tile([128, KC, BLK], bf16)
            for kc in range(KC):
                ps = psum.tile([128, BLK], f32, tag="ps")
                for jc in range(KC):
                    nc.tensor.matmul(ps[:], w_sb[:, jc, kc * 128:(kc + 1) * 128],
                                     zb_sb[:, jc, :],
                                     start=(jc == 0), stop=(jc == KC - 1))
                nc.scalar.copy(a_sb[:, kc, :], ps[:])

            # S block = A @ U^T, streamed to DRAM in [128, 512] tiles
            store_insts = []
            for mt in range(MT):
                for nch in range(NCH):
                    ps = psum.tile([128, 512], f32, tag="ps")
                    for kc in range(KC):
                        nc.tensor.matmul(
                            ps[:], a_sb[:, kc, mt * 128:(mt + 1) * 128],
                            zt_sb[:, kc, nch * 512:(nch + 1) * 512],
                            start=(kc == 0), stop=(kc == KC - 1))
                    s_sb = stage.tile([128, 512], bf16, tag="s_out")
                    nc.scalar.copy(s_sb[:], ps[:])
                    st = nc.sync.dma_start(
                        SD.ap()[mt * 128:(mt + 1) * 128, nch * 512:(nch + 1) * 512],
                        s_sb[:])
                    store_insts.append(st)

            # Per-edge extraction: gather 256B tokens (chunked so each
            # dma_gather fits the SWDGE descriptor ring), one-hot mask,
            # segmented reduce, sigmoid.
            ix_sb = sml.tile([128, cap // 16], i16)
            nc.sync.dma_start(ix_sb[:], IX.ap())
            ms_sb = big.tile([128, nblk, 128], bf16)
            if ms_load:
                nc.sync.dma_start(ms_sb[:], MS.ap())
            else:
                nc.gpsimd.memset(ms_sb[:], 1.0)
            g_sb = big.tile([128, nblk, 128], bf16)
            y_sb = sml.tile([128, nblk], f32)
            sd_view = SD.ap().rearrange("r (b c) -> (r b) c", c=128)
            p_sb = g_sb if inplace else big.tile([128, nblk, 128], bf16)
            CHUNK = 32  # blocks per dma_gather = 4096 indices
            for b0 in range(0, nblk, CHUNK):
                b1 = min(b0 + CHUNK, nblk)
                nidx = (b1 - b0) * 128
                if gather_mode == "real":
                    # single_packet=False: packed-single-packet mode faults the
                    # engine above 1024 idxs (64 descriptors/engine ceiling)
                    gi = nc.gpsimd.dma_gather(
                        g_sb[:, b0:b1, :], sd_view,
                        ix_sb[:, b0 * 8:b1 * 8],
                        num_idxs=nidx, num_idxs_reg=nidx, elem_size=128,
                        single_packet=False)
                    if dep_mode == "helper":
                        for st in store_insts:
                            add_dep_helper(gi.ins, st.ins,
                                           reason="gather reads S scratch")
                else:
                    nc.gpsimd.memset(g_sb[:, b0:b1, :], 0.5)
                if not tail:
                    continue
                nc.vector.tensor_tensor(p_sb[:, b0:b1, :], g_sb[:, b0:b1, :],
                                        ms_sb[:, b0:b1, :],
                                        op=mybir.AluOpType.mult)
                nc.vector.tensor_reduce(y_sb[:, b0:b1], p_sb[:, b0:b1, :],
                                        axis=mybir.AxisListType.X,
                                        op=mybir.AluOpType.add)
            o_sb = sml.tile([128, nblk], f32)
            if tail:
                nc.scalar.activation(o_sb[:], y_sb[:],
                                     mybir.ActivationFunctionType.Sigmoid)
            else:
                nc.vector.tensor_copy(o_sb[:], g_sb[:, :, 0])
            nc.sync.dma_start(OUT.ap(), o_sb[:])

    nc.compile()
    return nc


def _get_program(cap):
    if cap not in _cache:
        _cache[cap] = _build(cap)
    return _cache[cap]


def kernel(z_drug, global_weight, local_diag, batch_edges, edge_sub_type_idx,
           **_unused):
    from concourse.bass_utils import run_bass_kernel_spmd

    z = np.asarray(z_drug, np.float32)
    W = np.asarray(global_weight, np.float32)
    ld = np.asarray(local_diag, np.float32)
    e = np.asarray(batch_edges)
    sub = int(np.asarray(edge_sub_type_idx))
    d = ld[sub]
    assert z.shape == (N_DRUGS, D) and W.shape == (D, D)
    B = e.shape[1]
    e0 = e[0].astype(np.int64)
    e1 = e[1].astype(np.int64)

    zT = np.ascontiguousarray(z.T).astype(BF)          # [512, 4096]
    Wb = W.astype(BF)
    dT = np.ascontiguousarray(d.reshape(KC, 128).T)    # [128, 4] f32

    core = e0 // BLK
    counts = np.bincount(core, minlength=N_CORES)
    cap = max(128, int(-(-counts.max() // 128)) * 128)
    nblk = cap // 128

    in_maps = []
    positions = []
    one = BF(1.0)
    for c in range(N_CORES):
        sel = np.nonzero(core == c)[0]
        r = e0[sel] - c * BLK
        n = e1[sel]
        npad = cap - sel.size
        tok = np.zeros(cap, np.int16)
        tok[:sel.size] = (r * TPB + (n >> 7)).astype(np.int16)
        nm = np.zeros(cap, np.int64)
        nm[:sel.size] = n & 127
        # idx wrapped over 16 partitions, replicated to all 8 Q7 cores
        ixw = np.ascontiguousarray(
            np.tile(tok.reshape(cap // 16, 16).T, (8, 1)))
        mask = np.zeros((128, nblk, 128), BF)
        j = np.arange(cap)
        mask[j % 128, j // 128, nm] = one
        zB = np.ascontiguousarray(zT[:, c * BLK:(c + 1) * BLK])
        in_maps.append({"zt": zT, "zb": zB, "w": Wb, "dvec": dT,
                        "mask": mask, "idx": ixw})
        positions.append(sel)

    nc = _get_program(cap)
    res = run_bass_kernel_spmd(nc, in_maps, list(range(N_CORES)))

    out = np.empty(B, np.float32)
    for c in range(N_CORES):
        oc = np.asarray(res.results[c]["out"], np.float32)  # [128, nblk]
        flat = oc.T.reshape(-1)                             # j = b*128 + p
        out[positions[c]] = flat[:positions[c].size]
    return out


if __name__ == "__main__":
    dat = np.load("/root/problem/cached_io.npz")
    inputs = {k: dat[k] for k in ("z_drug", "global_weight", "local_diag",
                                  "batch_edges", "edge_sub_type_idx")}
    expected = dat["expected"]
    actual = kernel(**inputs)
    err = np.abs(actual - expected)
    print("max abs err:", err.max(), "mean:", err.mean())
    print("Relative error:", err.max() / np.abs(expected).max())


# revision 14
# speedup vs baseline: 1.0120x; 1.0120x over previous
"""Trainium2 Bass kernel for the Dedicom decoder problem.

Math: with U = z * d (row-wise scale by the selected local_diag row),
    score_b = ((z[e0]*d) @ W) * d . z[e1] = U[e0] @ W @ U[e1]^T
so all-pairs scores S = (U @ W) @ U^T  ([N_DRUGS, N_DRUGS]) contain every
edge score.  We shard S by e0-block across the 8 cores: core c computes
S rows [512c, 512c+512) (~2.1 GF in bf16), streams them to DRAM, then a
256B-granular dma_gather pulls each edge's 128-wide candidate block and a
host-built one-hot mask + segmented reduce extracts the scalar, followed
by an on-chip sigmoid.  Edges are bucketed to cores by e0>>9 on the host;
results are scattered back to their original positions on the host.
"""

import numpy as np
import ml_dtypes

BF = ml_dtypes.bfloat16

N_DRUGS = 4096
D = 512
N_CORES = 8
BLK = N_DRUGS // N_CORES  # 512 rows of S per core
KC = D // 128             # 4 contraction chunks
MT = BLK // 128           # 4 row tiles of the core's S block
NCH = N_DRUGS // 512      # 8 column chunks of S
TPB = N_DRUGS // 128      # 32 tokens (128-wide blocks) per S row

_cache = {}


def _build(cap, dep_mode="helper", inplace=True, tail=True, gather_mode="real",
           ms_load=True):
    """Build + compile the SPMD program for a per-core edge capacity `cap`."""
    import concourse.bass as bass  # noqa: F401
    import concourse.bacc as bacc
    import concourse.mybir as mybir
    import concourse.tile as tile
    from concourse.tile import add_dep_helper

    f32 = mybir.dt.float32
    bf16 = mybir.dt.bfloat16
    i16 = mybir.dt.int16
    nblk = cap // 128

    nc = bacc.Bacc("TRN2", target_bir_lowering=False, debug=False,
                   num_devices=N_CORES, dynamic_dma_scratch_size=65536)

    ZT = nc.dram_tensor("zt", [D, N_DRUGS], bf16, kind="ExternalInput")
    ZB = nc.dram_tensor("zb", [D, BLK], bf16, kind="ExternalInput")
    WT = nc.dram_tensor("w", [D, D], bf16, kind="ExternalInput")
    DT = nc.dram_tensor("dvec", [128, KC], f32, kind="ExternalInput")
    MS = nc.dram_tensor("mask", [128, nblk, 128], bf16, kind="ExternalInput")
    IX = nc.dram_tensor("idx", [128, cap // 16], i16, kind="ExternalInput")
    OUT = nc.dram_tensor("out", [128, nblk], f32, kind="ExternalOutput")
    SD = nc.dram_tensor("s_scratch", [BLK, N_DRUGS], bf16)

    with tile.TileContext(nc) as tc:
        with (
            tc.tile_pool(name="big", bufs=1) as big,
            tc.tile_pool(name="sml", bufs=1) as sml,
            tc.tile_pool(name="stage", bufs=8) as stage,
            tc.tile_pool(name="psum", bufs=8, space="PSUM") as psum,
        ):
            d_sb = sml.tile([128, KC], f32)
            nc.sync.dma_start(d_sb[:], DT.ap())
            w_sb = sml.tile([128, KC, D], bf16)
            nc.sync.dma_start(w_sb[:], WT.ap().rearrange("(jc p) k -> p jc k", p=128))
            zb_sb = sml.tile([128, KC, BLK], bf16)
            nc.sync.dma_start(zb_sb[:], ZB.ap().rearrange("(kc p) m -> p kc m", p=128))
            zt_sb = big.tile([128, KC, N_DRUGS], bf16)
            nc.sync.dma_start(zt_sb[:], ZT.ap().rearrange("(kc p) n -> p kc n", p=128))
            # issue extraction-phase inputs now: they ride the SP HWDGE FIFO
            # ahead of the S stores and transfer during the matmul phase
            ix_sb = sml.tile([128, cap // 16], i16)
            nc.sync.dma_start(ix_sb[:], IX.ap())
            ms_sb = big.tile([128, nblk, 128], bf16)
            if ms_load:
                nc.sync.dma_start(ms_sb[:], MS.ap())
            else:
                nc.gpsimd.memset(ms_sb[:], 1.0)

            # U^T = z^T * d  (d is a per-partition scalar in each K chunk)
            for kc in range(KC):
                nc.vector.tensor_scalar_mul(zb_sb[:, kc, :], zb_sb[:, kc, :],
                                            d_sb[:, kc:kc + 1])
                nc.vector.tensor_scalar_mul(zt_sb[:, kc, :], zt_sb[:, kc, :],
                                            d_sb[:, kc:kc + 1])

            # A^T chunks for this core's block: a_sb[p, kc, m] = (U@W)[m, kc*128+p]
            a_sb = sml.tile([128, KC, BLK], bf16)
            for kc in range(KC):
                ps = psum.tile([128, BLK], f32, tag="ps")
                for jc in range(KC):
                    nc.tensor.matmul(ps[:], w_sb[:, jc, kc * 128:(kc + 1) * 128],
                                     zb_sb[:, jc, :],
                                     start=(jc == 0), stop=(jc == KC - 1))
                nc.scalar.copy(a_sb[:, kc, :], ps[:])

            # S block = A @ U^T, streamed to DRAM in [128, 512] tiles.
            # kc-outer over 8 PSUM banks: each lhsT slice streams 8 moving
            # tiles, cutting PE weight-reload overhead. Casts split ACT/DVE;
            # stores ride the ACT HWDGE ring, separate from the input loads.
            store_insts = []
            for mt in range(MT):
                pss = [psum.tile([128, 512], f32, tag="ps", name=f"ps_{mt}_{i}")
                       for i in range(NCH)]
                for kc in range(KC):
                    for nch in range(NCH):
                        nc.tensor.matmul(
                            pss[nch][:], a_sb[:, kc, mt * 128:(mt + 1) * 128],
                            zt_sb[:, kc, nch * 512:(nch + 1) * 512],
                            start=(kc == 0), stop=(kc == KC - 1))
                for nch in range(NCH):
                    s_sb = stage.tile([128, 512], bf16, tag="s_out")
                    if nch % 2 == 0:
                        nc.scalar.copy(s_sb[:], pss[nch][:])
                    else:
                        nc.vector.tensor_copy(s_sb[:], pss[nch][:])
                    st = nc.scalar.dma_start(
                        SD.ap()[mt * 128:(mt + 1) * 128, nch * 512:(nch + 1) * 512],
                        s_sb[:])
                    store_insts.append(st)

            # Per-edge extraction: gather 256B tokens (chunked so each
            # dma_gather fits the SWDGE descriptor ring), one-hot mask,
            # segmented reduce, sigmoid.
            g_sb = big.tile([128, nblk, 128], bf16)
            y_sb = sml.tile([128, nblk], f32)
            sd_view = SD.ap().rearrange("r (b c) -> (r b) c", c=128)
            p_sb = g_sb if inplace else big.tile([128, nblk, 128], bf16)
            CHUNK = 32  # blocks per dma_gather = 4096 indices
            for b0 in range(0, nblk, CHUNK):
                b1 = min(b0 + CHUNK, nblk)
                nidx = (b1 - b0) * 128
                if gather_mode == "real":
                    # single_packet=False: packed-single-packet mode faults the
                    # engine above 1024 idxs (64 descriptors/engine ceiling)
                    gi = nc.gpsimd.dma_gather(
                        g_sb[:, b0:b1, :], sd_view,
                        ix_sb[:, b0 * 8:b1 * 8],
                        num_idxs=nidx, num_idxs_reg=nidx, elem_size=128,
                        single_packet=False)
                    if dep_mode == "helper":
                        for st in store_insts:
                            add_dep_helper(gi.ins, st.ins,
                                           reason="gather reads S scratch")
                else:
                    nc.gpsimd.memset(g_sb[:, b0:b1, :], 0.5)
                if not tail:
                    continue
                nc.vector.tensor_tensor(p_sb[:, b0:b1, :], g_sb[:, b0:b1, :],
                                        ms_sb[:, b0:b1, :],
                                        op=mybir.AluOpType.mult)
                nc.vector.tensor_reduce(y_sb[:, b0:b1], p_sb[:, b0:b1, :],
                                        axis=mybir.AxisListType.X,
                                        op=mybir.AluOpType.add)
            o_sb = sml.tile([128, nblk], f32)
            if tail:
                nc.scalar.activation(o_sb[:], y_sb[:],
                                     mybir.ActivationFunctionType.Sigmoid)
            else:
                nc.vector.tensor_copy(o_sb[:], g_sb[:, :, 0])
            nc.sync.dma_start(OUT.ap(), o_sb[:])

    nc.compile()
    return nc


def _get_program(cap):
    if cap not in _cache:
        _cache[cap] = _build(cap)
    return _cache[cap]


def kernel(z_drug, global_weight, local_diag, batch_edges, edge_sub_type_idx,
           **_unused):
    from concourse.bass_utils import run_bass_kernel_spmd

    z = np.asarray(z_drug, np.float32)
    W = np.asarray(global_weight, np.float32)
    ld = np.asarray(local_diag, np.float32)
    e = np.asarray(batch_edges)
    sub = int(np.asarray(edge_sub_type_idx))
    d = ld[sub]
    assert z.shape == (N_DRUGS, D) and W.shape == (D, D)
    B = e.shape[1]
    e0 = e[0].astype(np.int64)
    e1 = e[1].astype(np.int64)

    zT = np.ascontiguousarray(z.T).astype(BF)          # [512, 4096]
    Wb = W.astype(BF)
    dT = np.ascontiguousarray(d.reshape(KC, 128).T)    # [128, 4] f32

    core = e0 // BLK
    counts = np.bincount(core, minlength=N_CORES)
    cap = max(128, int(-(-counts.max() // 128)) * 128)
    nblk = cap // 128

    in_maps = []
    positions = []
    one = BF(1.0)
    for c in range(N_CORES):
        sel = np.nonzero(core == c)[0]
        r = e0[sel] - c * BLK
        n = e1[sel]
        npad = cap - sel.size
        tok = np.zeros(cap, np.int16)
        tok[:sel.size] = (r * TPB + (n >> 7)).astype(np.int16)
        nm = np.zeros(cap, np.int64)
        nm[:sel.size] = n & 127
        # idx wrapped over 16 partitions, replicated to all 8 Q7 cores
        ixw = np.ascontiguousarray(
            np.tile(tok.reshape(cap // 16, 16).T, (8, 1)))
        mask = np.zeros((128, nblk, 128), BF)
        j = np.arange(cap)
        mask[j % 128, j // 128, nm] = one
        zB = np.ascontiguousarray(zT[:, c * BLK:(c + 1) * BLK])
        in_maps.append({"zt": zT, "zb": zB, "w": Wb, "dvec": dT,
                        "mask": mask, "idx": ixw})
        positions.append(sel)

    nc = _get_program(cap)
    res = run_bass_kernel_spmd(nc, in_maps, list(range(N_CORES)))

    out = np.empty(B, np.float32)
    for c in range(N_CORES):
        oc = np.asarray(res.results[c]["out"], np.float32)  # [128, nblk]
        flat = oc.T.reshape(-1)                             # j = b*128 + p
        out[positions[c]] = flat[:positions[c].size]
    return out


if __name__ == "__main__":
    dat = np.load("/root/problem/cached_io.npz")
    inputs = {k: dat[k] for k in ("z_drug", "global_weight", "local_diag",
                                  "batch_edges", "edge_sub_type_idx")}
    expected = dat["expected"]
    actual = kernel(**inputs)
    err = np.abs(actual - expected)
    print("max abs err:", err.max(), "mean:", err.mean())
    print("Relative error:", err.max() / np.abs(expected).max())
